# revision 6
# baseline (speedup 1.0000x reference)
"""AttentionPool (segment softmax-pool) Trainium2 kernel.

Math (matches reference up to per-segment-constant invariance of softmax):
    h    = relu(x @ W1 + b1)                [N, 64]
    gate = h @ W2 (+ b2, dropped: constant) [N]
    alpha = segment_softmax(gate, batch)    [N]   (max-subtraction dropped:
                                                   gate is O(1), exp safe)
    out[g] = sum_{batch[i]==g} alpha[i] * x[i]    [G, 128]

Device strategy (per core, nodes split evenly across 8 cores):
  stream 512-node supertiles; per supertile:
    DMA x [128,4,128] natural -> 4x PE transpose -> xT [128,512]
    mm1: W1.T-style (lhsT=W1, rhs=xT) -> hT [64,512] psum
    ACT relu+bias -> h sbuf
    mm2 x4: lhsT=h-slice [64,128], rhs=W2 [64,1] -> gate COLUMN [128,1]
    ACT exp on gate [128,4] -> e
    DVE: E = host_mask(u8) * e  (E[n,j] = e_n iff node n belongs to j-th
         distinct graph of this supertile)
    ph2: accumulate psum [NG,129] over k: E_k.T @ x_k  and  E_k.T @ ones
    DMA psum partial -> DRAM [T,NG,129]
  Host scatter-adds per-supertile partials into [G,129] and divides.
"""

import numpy as np
from contextlib import ExitStack

import concourse.bass as bass
import concourse.tile as tile
from concourse import bacc, mybir
from concourse.bass_utils import run_bass_kernel_spmd
from concourse.masks import make_identity

F32 = mybir.dt.float32
U8 = mybir.dt.uint8

CORES = 8
D = 128
HID = 64
G_SEGMENTS = 8192
SUB = 128
KSUB = 4
SUPER = SUB * KSUB  # 512


def build_program(T: int, NG: int):
    """Build the per-core Bass program (same program for all 8 cores)."""
    nc = bacc.Bacc(None, target_bir_lowering=False)

    x_d = nc.dram_tensor("x", [T * SUPER, D], F32, kind="ExternalInput")
    mask_d = nc.dram_tensor("mask", [T, SUB, KSUB, NG], U8, kind="ExternalInput")
    w1_d = nc.dram_tensor("w1", [D, HID], F32, kind="ExternalInput")
    b1_d = nc.dram_tensor("b1", [HID, 1], F32, kind="ExternalInput")
    w2_d = nc.dram_tensor("w2", [HID, 1], F32, kind="ExternalInput")
    out_d = nc.dram_tensor("out_part", [T, NG, D + 1], F32, kind="ExternalOutput")

    x_r = x_d[:, :].rearrange("(t k p) d -> t p k d", k=KSUB, p=SUB)

    with ExitStack() as ctx:
        tc = ctx.enter_context(tile.TileContext(nc))
        consts = ctx.enter_context(tc.tile_pool(name="consts", bufs=1))
        xpool = ctx.enter_context(tc.tile_pool(name="xpool", bufs=3))
        xtpool = ctx.enter_context(tc.tile_pool(name="xtpool", bufs=2))
        hpool = ctx.enter_context(tc.tile_pool(name="hpool", bufs=2))
        epool = ctx.enter_context(tc.tile_pool(name="epool", bufs=2))
        opool = ctx.enter_context(tc.tile_pool(name="opool", bufs=3))
        mpool = ctx.enter_context(tc.tile_pool(name="mpool", bufs=3))
        ps_xt = ctx.enter_context(
            tc.tile_pool(name="ps_xt", bufs=2, space=bass.MemorySpace.PSUM)
        )
        ps_h = ctx.enter_context(
            tc.tile_pool(name="ps_h", bufs=2, space=bass.MemorySpace.PSUM)
        )
        ps_g = ctx.enter_context(
            tc.tile_pool(name="ps_g", bufs=2, space=bass.MemorySpace.PSUM)
        )
        ps_o = ctx.enter_context(
            tc.tile_pool(name="ps_o", bufs=2, space=bass.MemorySpace.PSUM)
        )

        ident = consts.tile([128, 128], F32)
        make_identity(nc, ident)
        w1 = consts.tile([D, HID], F32)
        nc.sync.dma_start(w1, w1_d[:, :])
        b1 = consts.tile([HID, 1], F32)
        nc.sync.dma_start(b1, b1_d[:, :])
        w2 = consts.tile([HID, 1], F32)
        nc.sync.dma_start(w2, w2_d[:, :])
        ones = consts.tile([128, 1], F32)
        nc.vector.memset(ones, 1.0)

        for t in range(T):
            x_sb = xpool.tile([SUB, KSUB, D], F32)
            nc.sync.dma_start(x_sb, x_r[t])
            m_sb = mpool.tile([SUB, KSUB, NG], U8)
            nc.sync.dma_start(m_sb, mask_d[t])

            pxt = ps_xt.tile([128, SUPER], F32)
            for k in range(KSUB):
                nc.tensor.transpose(
                    pxt[:, k * SUB : (k + 1) * SUB], x_sb[:, k, :], ident
                )
            xt = xtpool.tile([128, SUPER], F32)
            nc.vector.tensor_copy(xt, pxt)

            ph = ps_h.tile([HID, SUPER], F32)
            nc.tensor.matmul(ph, w1, xt, start=True, stop=True)
            h = hpool.tile([HID, SUPER], F32)
            nc.scalar.activation(
                h, ph, mybir.ActivationFunctionType.Relu, bias=b1, scale=1.0
            )

            pg = ps_g.tile([SUB, KSUB], F32)
            for k in range(KSUB):
                nc.tensor.matmul(
                    pg[:, k : k + 1],
                    h[:, k * SUB : (k + 1) * SUB],
                    w2,
                    start=True,
                    stop=True,
                )
            e = epool.tile([SUB, KSUB], F32)
            nc.scalar.activation(e, pg, mybir.ActivationFunctionType.Exp)

            E = epool.tile([SUB, KSUB, NG], F32)
            for k in range(KSUB):
                nc.vector.tensor_scalar_mul(E[:, k, :], m_sb[:, k, :], e[:, k : k + 1])

            po = ps_o.tile([NG, D + 1], F32)
            for k in range(KSUB):
                nc.tensor.matmul(
                    po[:, 0:D],
                    E[:, k, :],
                    x_sb[:, k, :],
                    start=(k == 0),
                    stop=(k == KSUB - 1),
                )
            for k in range(KSUB):
                nc.tensor.matmul(
                    po[:, D : D + 1],
                    E[:, k, :],
                    ones,
                    start=(k == 0),
                    stop=(k == KSUB - 1),
                )
            po_sb = opool.tile([NG, D + 1], F32)
            nc.scalar.copy(po_sb, po)
            nc.sync.dma_start(out_d[t], po_sb)

    nc.compile()
    return nc


def preprocess(x: np.ndarray, batch: np.ndarray):
    """Shard + pad inputs, build per-supertile masks and graph-id tables."""
    N = x.shape[0]
    n_core = -(-N // CORES)
    npc = -(-n_core // SUPER) * SUPER
    T = npc // SUPER

    xs = np.zeros((CORES, npc, D), np.float32)
    b_pad = np.empty((CORES, npc), np.int64)
    valid = np.zeros((CORES, npc), bool)
    for c in range(CORES):
        s, e = c * n_core, min((c + 1) * n_core, N)
        n = e - s
        xs[c, :n] = x[s:e]
        b_pad[c, :n] = batch[s:e]
        b_pad[c, n:] = batch[e - 1] if n > 0 else 0
        valid[c, :n] = True

    v = b_pad.reshape(CORES, T, SUPER)
    chg = np.zeros(v.shape, bool)
    chg[..., 1:] = v[..., 1:] != v[..., :-1]
    loc = np.cumsum(chg, axis=-1)  # [C,T,SUPER] local distinct index
    NG = int(loc.max()) + 1
    NG = max(4, -(-NG // 4) * 4)

    vmask = valid.reshape(CORES, T, SUPER)
    onehot = (loc[..., None] == np.arange(NG)) & vmask[..., None]
    # [C,T,SUPER,NG] -> [C,T,SUB,KSUB,NG]
    mask = np.ascontiguousarray(
        onehot.reshape(CORES, T, KSUB, SUB, NG).transpose(0, 1, 3, 2, 4)
    ).astype(np.uint8)

    gids = np.full((CORES, T, NG), -1, np.int64)
    cc, tt = np.meshgrid(np.arange(CORES), np.arange(T), indexing="ij")
    cc = cc[..., None] * np.ones((1, 1, SUPER), int)
    tt = tt[..., None] * np.ones((1, 1, SUPER), int)
    gids[cc.ravel(), tt.ravel(), loc.ravel()] = np.where(vmask, v, -1).ravel()

    return xs, mask, gids, T, NG


def _kernel_impl(x, batch, W1, b1, W2, b2=None, **run_kwargs):
    x = np.ascontiguousarray(np.asarray(x, dtype=np.float32))
    batch = np.asarray(batch).astype(np.int64)
    W1 = np.asarray(W1, dtype=np.float32)
    b1 = np.asarray(b1, dtype=np.float32).reshape(HID, 1)
    W2 = np.asarray(W2, dtype=np.float32).reshape(HID, 1)

    xs, mask, gids, T, NG = preprocess(x, batch)

    nc = build_program(T, NG)
    in_maps = [
        {
            "x": xs[c].reshape(T * SUPER, D),
            "mask": mask[c],
            "w1": W1,
            "b1": b1,
            "w2": W2,
        }
        for c in range(CORES)
    ]
    res = run_bass_kernel_spmd(nc, in_maps, core_ids=list(range(CORES)), **run_kwargs)
    parts = np.stack([r["out_part"] for r in res.results])  # [C,T,NG,129]

    G = G_SEGMENTS
    acc = np.zeros((G + 1, D + 1), np.float32)
    idx = np.where(gids >= 0, gids, G).ravel()
    np.add.at(acc, idx, parts.reshape(-1, D + 1))
    den = acc[:G, D]
    S = acc[:G, :D]
    out = np.where(den[:, None] > 0, S / np.maximum(den, 1e-30)[:, None], 0.0)
    return out.astype(np.float32), res


def kernel(x, batch, W1, b1, W2, b2):
    out, _ = _kernel_impl(x, batch, W1, b1, W2, b2)
    return out


# revision 8
# speedup vs baseline: 1.5692x; 1.5692x over previous
"""AttentionPool (segment softmax-pool) Trainium2 kernel.

Math (matches reference up to per-segment-constant invariance of softmax):
    h    = relu(x @ W1 + b1)                [N, 64]
    gate = h @ W2 (+ b2, dropped: constant) [N]
    alpha = segment_softmax(gate, batch)    [N]   (max-subtraction dropped:
                                                   gate is O(1), exp safe)
    out[g] = sum_{batch[i]==g} alpha[i] * x[i]    [G, 128]

Precision strategy: PE fp32 matmuls run as LOW/HIGH double passes (~4x
slower than bf16), so x is split on the host into x_hi + x_lo (both bf16;
same total DMA bytes as fp32). The gate MLP runs on x_hi in bf16; the
phase-2 weighted sum accumulates E.T @ x_hi + E.T @ x_lo in fp32 PSUM,
recovering ~fp32 output accuracy at bf16 speed.

Device pipeline per 512-node supertile (per core, nodes split across 8):
    DMA x_hi/x_lo [128,4,129] (col 128 = ones in hi, zeros in lo)
    4x PE transpose (bf16) -> xT [128,512] (DVE copy from PSUM)
    mm1: lhsT=W1 [128,64], rhs=xT -> hT [64,512] psum
    ACT relu+bias -> h bf16
    mm2 x4: lhsT=h-slice [64,128], rhs=W2 [64,1] -> gate COLUMN [128,1]
    ACT exp on gate [128,4] -> e (f32)
    DVE: E(bf16) = host_mask(u8) * e   (E[n,j] = e_n iff node n in j-th
         distinct graph of the supertile)
    ph2: psum [NG,129] += E_k.T @ xhi_k  and  += E_k.T @ xlo_k  (k=0..3)
    ACT copy psum -> SBUF, DMA -> DRAM partials [T,NG,129]
Host scatter-adds partials into [G,129] and divides.
"""

import numpy as np
from contextlib import ExitStack

import concourse.bass as bass
import concourse.tile as tile
from concourse import bacc, mybir
from concourse.bass_utils import run_bass_kernel_spmd
from concourse.masks import make_identity

F32 = mybir.dt.float32
BF16 = mybir.dt.bfloat16
U8 = mybir.dt.uint8

CORES = 8
D = 128
HID = 64
G_SEGMENTS = 8192
SUB = 128
KSUB = 4
SUPER = SUB * KSUB  # 512
DW = D + 1  # x row + ones column


def build_program(T: int, NG: int):
    """Build the per-core Bass program (same program for all 8 cores)."""
    nc = bacc.Bacc(None, target_bir_lowering=False)

    xhi_d = nc.dram_tensor("xhi", [T, SUB, KSUB, DW], BF16, kind="ExternalInput")
    xlo_d = nc.dram_tensor("xlo", [T, SUB, KSUB, DW], BF16, kind="ExternalInput")
    mask_d = nc.dram_tensor("mask", [T, SUB, KSUB, NG], U8, kind="ExternalInput")
    w1_d = nc.dram_tensor("w1", [D, HID], BF16, kind="ExternalInput")
    b1_d = nc.dram_tensor("b1", [HID, 1], F32, kind="ExternalInput")
    w2_d = nc.dram_tensor("w2", [HID, 1], BF16, kind="ExternalInput")
    out_d = nc.dram_tensor("out_part", [T, NG, DW], F32, kind="ExternalOutput")

    with ExitStack() as ctx:
        tc = ctx.enter_context(tile.TileContext(nc))
        consts = ctx.enter_context(tc.tile_pool(name="consts", bufs=1))
        xpool = ctx.enter_context(tc.tile_pool(name="xpool", bufs=3))
        xtpool = ctx.enter_context(tc.tile_pool(name="xtpool", bufs=2))
        hpool = ctx.enter_context(tc.tile_pool(name="hpool", bufs=2))
        epool = ctx.enter_context(tc.tile_pool(name="epool", bufs=2))
        opool = ctx.enter_context(tc.tile_pool(name="opool", bufs=3))
        mpool = ctx.enter_context(tc.tile_pool(name="mpool", bufs=3))
        ps_xt = ctx.enter_context(
            tc.tile_pool(name="ps_xt", bufs=2, space=bass.MemorySpace.PSUM)
        )
        ps_h = ctx.enter_context(
            tc.tile_pool(name="ps_h", bufs=2, space=bass.MemorySpace.PSUM)
        )
        ps_g = ctx.enter_context(
            tc.tile_pool(name="ps_g", bufs=2, space=bass.MemorySpace.PSUM)
        )
        ps_o = ctx.enter_context(
            tc.tile_pool(name="ps_o", bufs=2, space=bass.MemorySpace.PSUM)
        )

        ident = consts.tile([128, 128], BF16)
        make_identity(nc, ident)
        w1 = consts.tile([D, HID], BF16)
        nc.sync.dma_start(w1, w1_d[:, :])
        b1 = consts.tile([HID, 1], F32)
        nc.sync.dma_start(b1, b1_d[:, :])
        w2 = consts.tile([HID, 1], BF16)
        nc.sync.dma_start(w2, w2_d[:, :])

        for t in range(T):
            xhi = xpool.tile([SUB, KSUB, DW], BF16, tag="xhi")
            nc.sync.dma_start(xhi, xhi_d[t])
            xlo = xpool.tile([SUB, KSUB, DW], BF16, tag="xlo")
            nc.sync.dma_start(xlo, xlo_d[t])
            m_sb = mpool.tile([SUB, KSUB, NG], U8)
            nc.sync.dma_start(m_sb, mask_d[t])

            pxt = ps_xt.tile([128, SUPER], BF16)
            for k in range(KSUB):
                nc.tensor.transpose(
                    pxt[:, k * SUB : (k + 1) * SUB], xhi[:, k, 0:D], ident
                )
            xt = xtpool.tile([128, SUPER], BF16)
            nc.vector.tensor_copy(xt, pxt)

            ph = ps_h.tile([HID, SUPER], F32)
            nc.tensor.matmul(ph, w1, xt, start=True, stop=True)
            h = hpool.tile([HID, SUPER], BF16)
            nc.scalar.activation(
                h, ph, mybir.ActivationFunctionType.Relu, bias=b1, scale=1.0
            )

            pg = ps_g.tile([SUB, KSUB], F32)
            for k in range(KSUB):
                nc.tensor.matmul(
                    pg[:, k : k + 1],
                    h[:, k * SUB : (k + 1) * SUB],
                    w2,
                    start=True,
                    stop=True,
                )
            e = epool.tile([SUB, KSUB], F32, tag="e")
            nc.scalar.activation(e, pg, mybir.ActivationFunctionType.Exp)

            E = epool.tile([SUB, KSUB, NG], BF16, tag="E")
            for k in range(KSUB):
                nc.vector.tensor_scalar_mul(E[:, k, :], m_sb[:, k, :], e[:, k : k + 1])

            po = ps_o.tile([NG, DW], F32)
            for k in range(KSUB):
                nc.tensor.matmul(
                    po, E[:, k, :], xhi[:, k, :], start=(k == 0), stop=False
                )
                nc.tensor.matmul(
                    po, E[:, k, :], xlo[:, k, :], start=False, stop=(k == KSUB - 1)
                )
            po_sb = opool.tile([NG, DW], F32)
            nc.scalar.copy(po_sb, po)
            nc.sync.dma_start(out_d[t], po_sb)

    nc.compile()
    return nc


def preprocess(x: np.ndarray, batch: np.ndarray):
    """Shard + pad inputs, split x into bf16 hi/lo in device layout,
    build per-supertile masks and graph-id tables."""
    import ml_dtypes

    N = x.shape[0]
    n_core = -(-N // CORES)
    npc = -(-n_core // SUPER) * SUPER
    T = npc // SUPER

    xs = np.zeros((CORES, npc, D), np.float32)
    b_pad = np.empty((CORES, npc), np.int64)
    valid = np.zeros((CORES, npc), bool)
    for c in range(CORES):
        s, e = c * n_core, min((c + 1) * n_core, N)
        n = e - s
        xs[c, :n] = x[s:e]
        b_pad[c, :n] = batch[s:e]
        b_pad[c, n:] = batch[e - 1] if n > 0 else 0
        valid[c, :n] = True

    # hi/lo bf16 split in device layout [C,T,SUB,KSUB,DW]
    bf = ml_dtypes.bfloat16
    xhi = np.zeros((CORES, T, SUB, KSUB, DW), bf)
    xlo = np.zeros((CORES, T, SUB, KSUB, DW), bf)
    x4 = xs.reshape(CORES, T, KSUB, SUB, D).transpose(0, 1, 3, 2, 4)  # [C,T,SUB,KSUB,D]
    hi = x4.astype(bf)
    xhi[..., :D] = hi
    xhi[..., D] = bf(1.0)
    xlo[..., :D] = (x4 - hi.astype(np.float32)).astype(bf)

    v = b_pad.reshape(CORES, T, SUPER)
    chg = np.zeros(v.shape, bool)
    chg[..., 1:] = v[..., 1:] != v[..., :-1]
    loc = np.cumsum(chg, axis=-1)  # [C,T,SUPER] local distinct index
    NG = int(loc.max()) + 1
    NG = max(4, -(-NG // 4) * 4)

    vmask = valid.reshape(CORES, T, SUPER)
    onehot = (loc[..., None] == np.arange(NG)) & vmask[..., None]
    # [C,T,SUPER,NG] -> [C,T,SUB,KSUB,NG]
    mask = np.ascontiguousarray(
        onehot.reshape(CORES, T, KSUB, SUB, NG).transpose(0, 1, 3, 2, 4)
    ).astype(np.uint8)

    gids = np.full((CORES, T, NG), -1, np.int64)
    cc, tt = np.meshgrid(np.arange(CORES), np.arange(T), indexing="ij")
    cc = cc[..., None] * np.ones((1, 1, SUPER), int)
    tt = tt[..., None] * np.ones((1, 1, SUPER), int)
    gids[cc.ravel(), tt.ravel(), loc.ravel()] = np.where(vmask, v, -1).ravel()

    return xhi, xlo, mask, gids, T, NG


def _kernel_impl(x, batch, W1, b1, W2, b2=None, **run_kwargs):
    import ml_dtypes

    bf = ml_dtypes.bfloat16
    x = np.ascontiguousarray(np.asarray(x, dtype=np.float32))
    batch = np.asarray(batch).astype(np.int64)
    W1 = np.asarray(W1, dtype=np.float32).astype(bf)
    b1 = np.asarray(b1, dtype=np.float32).reshape(HID, 1)
    W2 = np.asarray(W2, dtype=np.float32).astype(bf).reshape(HID, 1)

    xhi, xlo, mask, gids, T, NG = preprocess(x, batch)

    nc = build_program(T, NG)
    in_maps = [
        {
            "xhi": xhi[c],
            "xlo": xlo[c],
            "mask": mask[c],
            "w1": W1,
            "b1": b1,
            "w2": W2,
        }
        for c in range(CORES)
    ]
    res = run_bass_kernel_spmd(nc, in_maps, core_ids=list(range(CORES)), **run_kwargs)
    parts = np.stack([r["out_part"] for r in res.results])  # [C,T,NG,DW]

    G = G_SEGMENTS
    acc = np.zeros((G + 1, DW), np.float32)
    idx = np.where(gids >= 0, gids, G).ravel()
    np.add.at(acc, idx, parts.reshape(-1, DW))
    den = acc[:G, D]
    S = acc[:G, :D]
    out = np.where(den[:, None] > 0, S / np.maximum(den, 1e-30)[:, None], 0.0)
    return out.astype(np.float32), res


def kernel(x, batch, W1, b1, W2, b2):
    out, _ = _kernel_impl(x, batch, W1, b1, W2, b2)
    return out
